# revision 10
# baseline (speedup 1.0000x reference)
"""NexusNet Trainium2 kernel (8-core SPMD, Bass/Tile).

Self-contained: only imports numpy + jax (for bf16 cast) + concourse.

Strategy (graph/data parallel per the sharding hint): two SPMD launches.
  L1 (dst-partition): each core owns M/8 nexus rows. Per-edge dma_gather of
     x[src] rows (bf16), one-hot segment-sum matmuls into PSUM windows,
     nexus MLP, then per-nexus-row linear precomputes:
        an = W1n_edge^T n   (edge-MLP n-side term, per plane)
        z  = W1n_node^T n   (node-MLP n-side term, per plane)
     written as packed rows [an | z] (768 bf16 = 1536B) per plane.
  Host concatenates the 8 andz slices (index-space stitch only).
  L2 (src-partition): each core owns N/8 plane nodes + their edges.
     Prologue per plane: ax = x W1x_edge + b1 (per node), axn = x W1x_node + b1n.
     Per edge tile: gather ax[src], [an|z][dst]; h = tanh(ax+an);
     logits = reduce(h*w2)+b2; softmax over classes; msg = w * z;
     weighted one-hot segment matmul -> agg per 128-node block;
     u1 = tanh(axn + invcnt*agg); u2 = tanh(u1 W2 + b2n) -> output.
"""

import numpy as np

from concourse import bacc, bass, mybir, tile
from concourse.bass_utils import run_bass_kernel_spmd

DT = mybir.dt
BF = DT.bfloat16
F32 = DT.float32
I16 = DT.int16
AF = mybir.ActivationFunctionType
OP = mybir.AluOpType


def ceil_div(a, b):
    return (a + b - 1) // b


class Cfg:
    def __init__(self, N=80000, M=40000, E=120000, ncores=8,
                 chunk=20000, nwh=20, bg=8, dch=None):
        self.N, self.M, self.E = N, M, E
        self.C, self.F, self.S, self.EF, self.P = 5, 64, 64, 64, 3
        self.RB = 384          # padded bf16 row elems (C*F=320 -> 384; 768B)
        self.RB2 = 768         # [an|z] row elems (1536B)
        self.NC = ncores
        assert N % ncores == 0 and M % ncores == 0
        self.NK = N // ncores              # plane nodes per core
        self.MK = M // ncores              # nexus rows per core
        self.NW = ceil_div(self.MK, 128)   # L1 windows per core
        self.MKP = self.NW * 128           # padded andz rows per core
        self.NB = ceil_div(self.NK, 128)   # L2 node blocks per core
        self.NKP = self.NB * 128
        self.NT512 = ceil_div(self.NKP, 512)
        self.NKP512 = self.NT512 * 512
        self.CHUNK = min(chunk, N)         # L1 src chunk rows (int16 limit)
        self.NQ = ceil_div(N, self.CHUNK)
        self.HROWS = ncores * self.MKP     # stitched andz rows
        self.DCH = dch if dch is not None else (
            self.HROWS if self.HROWS <= 32767 else ceil_div(self.HROWS, 2))
        self.NH = ceil_div(self.HROWS, self.DCH)
        assert self.CHUNK <= 32767 and self.DCH <= 32767 and self.NK <= 32767
        self.NWH = min(nwh, self.NW)       # windows per L1 half-pass
        self.NHALF = ceil_div(self.NW, self.NWH)
        self.BG = bg                       # L2 blocks per gather group
        self.GRP = [(0, 1), (2, 3), (4,)]  # class pair groups


def wrap_idx(idx):
    """[n] int array -> [128, n//16] int16 wrapped+replicated layout."""
    n = len(idx)
    assert n % 16 == 0
    w16 = np.asarray(idx, np.int16).reshape(n // 16, 16).T.copy()
    return np.tile(w16, (8, 1))


def blkdiag(mats):
    rs = sum(m.shape[0] for m in mats)
    cs = sum(m.shape[1] for m in mats)
    out = np.zeros((rs, cs), np.float32)
    r = c = 0
    for m in mats:
        out[r:r + m.shape[0], c:c + m.shape[1]] = m
        r += m.shape[0]
        c += m.shape[1]
    return out


def to_bf16(x):
    import ml_dtypes
    return np.asarray(x).astype(ml_dtypes.bfloat16)


def pack_rows_bf16(x2d, rb):
    n, d = x2d.shape
    out = np.zeros((n, rb), dtype=to_bf16(np.zeros(1)).dtype)
    out[:, :d] = to_bf16(x2d)
    return out


# ----------------------------------------------------------------------------
# Host preprocessing
# ----------------------------------------------------------------------------

class Prep:
    pass


def host_prep(cfg, inputs):
    c = cfg
    pr = Prep()
    planes = "uvy"

    xs = [np.asarray(inputs[f"x_{p}"], np.float32).reshape(c.N, c.C * c.F)
          for p in planes]
    edges = [np.asarray(inputs[f"edge_{p}"], np.int64) for p in planes]

    pr.xb = [pack_rows_bf16(x, c.RB) for x in xs]
    pr.xb_slice = [[None] * c.P for _ in range(c.NC)]
    for k in range(c.NC):
        for p in range(c.P):
            s = np.zeros((c.NKP512, c.RB), pr.xb[0].dtype)
            s[:c.NK] = pr.xb[p][k * c.NK:(k + 1) * c.NK]
            pr.xb_slice[k][p] = s

    # ---------------- L1 edge grouping: (plane, window, chunk) ----------
    l1_lists = [[[[None] * c.NQ for _ in range(c.NW)] for _ in range(c.P)]
                for _ in range(c.NC)]
    for p in range(c.P):
        src, dst = edges[p][0], edges[p][1]
        owner = dst // c.MK
        for k in range(c.NC):
            sel = owner == k
            s = src[sel]
            d = dst[sel] - k * c.MK
            wi = d // 128
            q = s // c.CHUNK
            for w in range(c.NW):
                for qq in range(c.NQ):
                    m = (wi == w) & (q == qq)
                    l1_lists[k][p][w][qq] = (s[m] - qq * c.CHUNK, d[m] - w * 128)

    pr.T1 = [[[0] * c.NQ for _ in range(c.NW)] for _ in range(c.P)]
    for p in range(c.P):
        for w in range(c.NW):
            for q in range(c.NQ):
                mx = max(len(l1_lists[k][p][w][q][0]) for k in range(c.NC))
                pr.T1[p][w][q] = ceil_div(mx, 128)

    # global tile order: wi asc, q asc, t ; chunk order: wi asc, t
    pr.tiles1 = []          # per p: list of (wi, q, pos_in_q)
    pr.qtiles1 = []
    for p in range(c.P):
        qpos = [0] * c.NQ
        tl = []
        for w in range(c.NW):
            for q in range(c.NQ):
                for t in range(pr.T1[p][w][q]):
                    tl.append((w, q, qpos[q]))
                    qpos[q] += 1
        pr.tiles1.append(tl)
        pr.qtiles1.append(qpos)

    pr.seg1 = [[[None] * c.NHALF for _ in range(c.NQ)] for _ in range(c.P)]
    for p in range(c.P):
        for q in range(c.NQ):
            pos = 0
            for hf in range(c.NHALF):
                w0, w1 = hf * c.NWH, min((hf + 1) * c.NWH, c.NW)
                n = sum(pr.T1[p][w][q] for w in range(w0, w1))
                pr.seg1[p][q][hf] = (pos, n)
                pos += n

    pr.gidx1 = [[[None] * c.NQ for _ in range(c.P)] for _ in range(c.NC)]
    pr.dloc1 = [[None] * c.P for _ in range(c.NC)]
    for k in range(c.NC):
        for p in range(c.P):
            ntile_tot = len(pr.tiles1[p])
            dl = np.full((128, max(ntile_tot, 1)), -1.0, np.float32)
            per_q = [np.zeros(max(pr.qtiles1[p][q], 1) * 128, np.int64)
                     for q in range(c.NQ)]
            consumed = {}
            for gti, (w, q, posq) in enumerate(pr.tiles1[p]):
                s_arr, d_arr = l1_lists[k][p][w][q]
                off = consumed.get((w, q), 0)
                take = s_arr[off:off + 128]
                dtake = d_arr[off:off + 128]
                consumed[(w, q)] = off + 128
                nslot = len(take)
                per_q[q][posq * 128:posq * 128 + nslot] = take
                dl[:nslot, gti] = dtake
            for q in range(c.NQ):
                pr.gidx1[k][p][q] = wrap_idx(per_q[q])
            pr.dloc1[k][p] = dl

    # ---------------- L2 edge grouping: (plane, block, hchunk) ----------
    l2_lists = [[[[None] * c.NH for _ in range(c.NB)] for _ in range(c.P)]
                for _ in range(c.NC)]
    pr.invcnt = [[None] * c.P for _ in range(c.NC)]
    for p in range(c.P):
        src, dst = edges[p][0], edges[p][1]
        owner = src // c.NK
        drow = (dst // c.MK) * c.MKP + (dst % c.MK)
        for k in range(c.NC):
            sel = owner == k
            s = src[sel] - k * c.NK
            dr = drow[sel]
            b = s // 128
            h = dr // c.DCH
            for bb in range(c.NB):
                for hh in range(c.NH):
                    m = (b == bb) & (h == hh)
                    l2_lists[k][p][bb][hh] = (s[m], dr[m] - hh * c.DCH,
                                              s[m] - bb * 128)
            deg = np.bincount(s, minlength=c.NKP).astype(np.float32)
            ic = 1.0 / np.maximum(deg, 1.0)
            pr.invcnt[k][p] = ic[:c.NKP].reshape(c.NB, 128).T.copy()

    pr.T2 = [[[0] * c.NH for _ in range(c.NB)] for _ in range(c.P)]
    for p in range(c.P):
        for b in range(c.NB):
            for h in range(c.NH):
                mx = max(len(l2_lists[k][p][b][h][0]) for k in range(c.NC))
                pr.T2[p][b][h] = ceil_div(mx, 128)

    pr.tiles2 = []          # per p: list of (b, h, pos_in_h)
    pr.htiles2 = []
    for p in range(c.P):
        hpos = [0] * c.NH
        tl = []
        for b in range(c.NB):
            for h in range(c.NH):
                for t in range(pr.T2[p][b][h]):
                    tl.append((b, h, hpos[h]))
                    hpos[h] += 1
        pr.tiles2.append(tl)
        pr.htiles2.append(hpos)

    pr.NBG = ceil_div(c.NB, c.BG)
    pr.seg2s = [[None] * pr.NBG for _ in range(c.P)]
    pr.seg2d = [[[None] * pr.NBG for _ in range(c.NH)] for _ in range(c.P)]
    for p in range(c.P):
        gpos = 0
        hseen = [0] * c.NH
        for g in range(pr.NBG):
            b0, b1 = g * c.BG, min((g + 1) * c.BG, c.NB)
            n = sum(pr.T2[p][b][h] for b in range(b0, b1) for h in range(c.NH))
            pr.seg2s[p][g] = (gpos, n)
            gpos += n
            for h in range(c.NH):
                nh = sum(pr.T2[p][b][h] for b in range(b0, b1))
                pr.seg2d[p][h][g] = (hseen[h], nh)
                hseen[h] += nh

    pr.gidx2s = [[None] * c.P for _ in range(c.NC)]
    pr.gidx2d = [[[None] * c.NH for _ in range(c.P)] for _ in range(c.NC)]
    pr.sloc2 = [[None] * c.P for _ in range(c.NC)]
    for k in range(c.NC):
        for p in range(c.P):
            ntile_tot = len(pr.tiles2[p])
            sl = np.full((128, max(ntile_tot, 1)), -1.0, np.float32)
            sg = np.zeros(max(ntile_tot, 1) * 128, np.int64)
            per_h = [np.zeros(max(pr.htiles2[p][h], 1) * 128, np.int64)
                     for h in range(c.NH)]
            consumed = {}
            for gti, (b, h, posh) in enumerate(pr.tiles2[p]):
                s_arr, d_arr, loc_arr = l2_lists[k][p][b][h]
                off = consumed.get((b, h), 0)
                take_s = s_arr[off:off + 128]
                take_d = d_arr[off:off + 128]
                take_l = loc_arr[off:off + 128]
                consumed[(b, h)] = off + 128
                nslot = len(take_s)
                sg[gti * 128:gti * 128 + nslot] = take_s
                per_h[h][posh * 128:posh * 128 + nslot] = take_d
                sl[:nslot, gti] = take_l
            pr.gidx2s[k][p] = wrap_idx(sg)
            for h in range(c.NH):
                pr.gidx2d[k][p][h] = wrap_idx(per_h[h])
            pr.sloc2[k][p] = sl

    # ---------------- weights packing ------------------------------------
    C, F = c.C, c.F
    nex_w1 = np.asarray(inputs["nex_w1"], np.float32)
    nex_b1 = np.asarray(inputs["nex_b1"], np.float32)
    nex_w2 = np.asarray(inputs["nex_w2"], np.float32)
    nex_b2 = np.asarray(inputs["nex_b2"], np.float32)
    edge_w1 = np.asarray(inputs["edge_w1"], np.float32)
    edge_b1 = np.asarray(inputs["edge_b1"], np.float32)
    edge_w2 = np.asarray(inputs["edge_w2"], np.float32)
    edge_b2 = np.asarray(inputs["edge_b2"], np.float32)
    node_w1 = np.asarray(inputs["node_w1"], np.float32)
    node_b1 = np.asarray(inputs["node_b1"], np.float32)
    node_w2 = np.asarray(inputs["node_w2"], np.float32)
    node_b2 = np.asarray(inputs["node_b2"], np.float32)

    G = c.GRP
    W = {}
    for gi, g in enumerate(G):
        for p in range(c.P):
            W[f"nexW1_{p}_{gi}"] = to_bf16(
                blkdiag([nex_w1[cc, p * F:(p + 1) * F, :] for cc in g]))
            W[f"anW_{p}_{gi}"] = to_bf16(
                blkdiag([edge_w1[p, cc, F:, :] for cc in g]))
            W[f"zW_{p}_{gi}"] = to_bf16(
                blkdiag([node_w1[p, cc, F:, :] for cc in g]))
            W[f"axW_{p}_{gi}"] = to_bf16(
                blkdiag([edge_w1[p, cc, :F, :] for cc in g]))
            W[f"axnW_{p}_{gi}"] = to_bf16(
                blkdiag([node_w1[p, cc, :F, :] for cc in g]))
            W[f"u2W_{p}_{gi}"] = to_bf16(
                blkdiag([node_w2[p, cc] for cc in g]))
            W[f"axB_{p}_{gi}"] = np.concatenate(
                [edge_b1[p, cc] for cc in g])[:, None].astype(np.float32)
            W[f"axnB_{p}_{gi}"] = np.concatenate(
                [node_b1[p, cc] for cc in g])[:, None].astype(np.float32)
            W[f"u2B_{p}_{gi}"] = np.concatenate(
                [node_b2[p, cc] for cc in g])[:, None].astype(np.float32)
        W[f"nexW2_{gi}"] = to_bf16(blkdiag([nex_w2[cc] for cc in g]))
        W[f"nexB1_{gi}"] = np.concatenate(
            [nex_b1[cc] for cc in g])[:, None].astype(np.float32)
        W[f"nexB2_{gi}"] = np.concatenate(
            [nex_b2[cc] for cc in g])[:, None].astype(np.float32)
    for p in range(c.P):
        W[f"w2rep_{p}"] = np.tile(
            to_bf16(edge_w2[p, :, :, 0].reshape(1, C * c.EF)), (128, 1))
        W[f"b2rep_{p}"] = np.tile(edge_b2[p, :, 0].reshape(1, C),
                                  (128, 1)).astype(np.float32)
    W["iota"] = np.tile(np.arange(128, dtype=np.float32), (128, 1))
    W["idbf"] = to_bf16(np.eye(128, dtype=np.float32))
    W["idf32"] = np.eye(128, dtype=np.float32)
    pr.W = W
    return pr


def _wdt(arr):
    return BF if arr.dtype != np.float32 else F32


GATHER_MAX_TILES = 8   # 1024 idxs: SWDGE descriptor ring holds only 1024


def emit_gather(nc, out_tile, in_ap, idx_tile, t0, ntiles, elem):
    """dma_gather of `ntiles`*128 rows, split into ring-sized sub-calls.

    out_tile[:, j, :] receives row j of the segment; idx columns start at
    tile t0 of idx_tile (8 int16 columns per tile).
    """
    done = 0
    while done < ntiles:
        n = min(GATHER_MAX_TILES, ntiles - done)
        nc.gpsimd.dma_gather(
            out_ap=out_tile[:, done:done + n, :],
            in_ap=in_ap,
            idxs_ap=idx_tile[:, (t0 + done) * 8:(t0 + done + n) * 8],
            num_idxs=n * 128,
            num_idxs_reg=n * 128,
            elem_size=elem,
        )
        done += n


# ----------------------------------------------------------------------------
# Launch 1 builder (nexus phase)
# ----------------------------------------------------------------------------

def build_l1(cfg, pr):
    c = cfg
    nc = bacc.Bacc("TRN2", target_bir_lowering=False, debug=False,
                   num_devices=c.NC)

    xb = [nc.dram_tensor(f"xb{p}", [c.N, c.RB], BF, kind="ExternalInput")
          for p in range(c.P)]
    gidx = [[nc.dram_tensor(f"gidx1_{p}_{q}",
                            list(pr.gidx1[0][p][q].shape), I16,
                            kind="ExternalInput")
             for q in range(c.NQ)] for p in range(c.P)]
    dloc = [nc.dram_tensor(f"dloc1_{p}", list(pr.dloc1[0][p].shape), F32,
                           kind="ExternalInput") for p in range(c.P)]

    wnames = ["iota", "idbf", "idf32"]
    for gi in range(len(c.GRP)):
        wnames += [f"nexW2_{gi}", f"nexB1_{gi}", f"nexB2_{gi}"]
        for p in range(c.P):
            wnames += [f"nexW1_{p}_{gi}", f"anW_{p}_{gi}", f"zW_{p}_{gi}"]
    wt = {n: nc.dram_tensor(n, list(pr.W[n].shape), _wdt(pr.W[n]),
                            kind="ExternalInput") for n in wnames}

    andz = nc.dram_tensor("andz", [c.P, c.MKP, c.RB2], BF,
                          kind="ExternalOutput")

    G = c.GRP
    with tile.TileContext(nc) as tc:
        with tc.tile_pool(name="const", bufs=1) as cpool, \
             tc.tile_pool(name="asb", bufs=1) as apool, \
             tc.tile_pool(name="gx", bufs=2) as gpool, \
             tc.tile_pool(name="work", bufs=3) as wpool, \
             tc.tile_pool(name="outp", bufs=3) as opool, \
             tc.tile_pool(name="psA", bufs=2, space="PSUM") as psA, \
             tc.tile_pool(name="psM", bufs=2, space="PSUM") as psM, \
             tc.tile_pool(name="psT", bufs=2, space="PSUM") as psT:

            cw = {}
            for n in wnames:
                t = cpool.tile(list(pr.W[n].shape), _wdt(pr.W[n]), tag=n)
                nc.sync.dma_start(out=t[:], in_=wt[n].ap())
                cw[n] = t
            cidx = {}
            for p in range(c.P):
                for q in range(c.NQ):
                    t = cpool.tile(list(pr.gidx1[0][p][q].shape), I16,
                                   tag=f"gi{p}_{q}")
                    nc.sync.dma_start(out=t[:], in_=gidx[p][q].ap())
                    cidx[(p, q)] = t
            cdl = {}
            for p in range(c.P):
                t = cpool.tile(list(pr.dloc1[0][p].shape), F32, tag=f"dl{p}")
                nc.sync.dma_start(out=t[:], in_=dloc[p].ap())
                cdl[p] = t

            A = [apool.tile([128, c.NWH, 320], F32, tag=f"A{p}", name=f"A{p}")
                 for p in range(c.P)]

            maxseg = max(max((pr.seg1[p][q][hf][1] for q in range(c.NQ)
                              for hf in range(c.NHALF)), default=1), 1)

            for hf in range(c.NHALF):
                w0 = hf * c.NWH
                w1 = min(w0 + c.NWH, c.NW)
                # ---- phase A: gathers + window accumulation ----
                for p in range(c.P):
                    for q in range(c.NQ):
                        t0, nseg = pr.seg1[p][q][hf]
                        if nseg == 0:
                            continue
                        gt = gpool.tile([128, maxseg, c.RB], BF, tag="gx")
                        emit_gather(
                            nc, gt,
                            xb[p].ap()[q * c.CHUNK:
                                       min((q + 1) * c.CHUNK, c.N), :],
                            cidx[(p, q)], t0, nseg, c.RB)
                        byw = {}
                        for gti, (wq, q_, posq) in enumerate(pr.tiles1[p]):
                            if q_ == q and w0 <= wq < w1:
                                byw.setdefault(wq, []).append((gti, posq))
                        for wq in sorted(byw):
                            tl = byw[wq]
                            aps = psA.tile([128, 320], F32, tag="Aps")
                            for j, (gti, posq) in enumerate(tl):
                                oh = wpool.tile([128, 128], BF, tag="oh")
                                nc.vector.tensor_tensor(
                                    out=oh[:],
                                    in0=cdl[p][:, gti:gti + 1]
                                        .to_broadcast([128, 128]),
                                    in1=cw["iota"][:],
                                    op=OP.is_equal)
                                nc.tensor.matmul(
                                    out=aps[:],
                                    lhsT=oh[:],
                                    rhs=gt[:, posq - t0, :320],
                                    start=(j == 0), stop=(j == len(tl) - 1))
                            dst = A[p][:, wq - w0, :]
                            first = all(pr.T1[p][wq][qq] == 0
                                        for qq in range(q))
                            if first:
                                nc.vector.tensor_copy(out=dst, in_=aps[:])
                            else:
                                nc.vector.tensor_tensor(
                                    out=dst, in0=dst, in1=aps[:], op=OP.add)
                for p in range(c.P):
                    for wq in range(w0, w1):
                        if all(pr.T1[p][wq][q] == 0 for q in range(c.NQ)):
                            nc.vector.memset(A[p][:, wq - w0, :], 0.0)

                # ---- phase B: per-window MLP ----
                for wq in range(w0, w1):
                    wl = wq - w0
                    at = []
                    for p in range(c.P):
                        atp = wpool.tile([128, 3, 128], BF, tag=f"at{p}")
                        for j in range(3):
                            cols = 320 - 128 * j if j == 2 else 128
                            tp = psT.tile([128, 128], F32, tag="tp")
                            nc.tensor.transpose(
                                out=tp[:cols, :128],
                                in_=A[p][:, wl, 128 * j:128 * j + cols],
                                identity=cw["idf32"][:])
                            nc.vector.tensor_copy(out=atp[:cols, j, :],
                                                  in_=tp[:cols, :128])
                        at.append(atp)
                    nT = []
                    for gi, g in enumerate(G):
                        gp = 64 * len(g)
                        m1 = psM.tile([128, 128], F32, tag="mlp")
                        for p in range(c.P):
                            nc.tensor.matmul(
                                out=m1[:gp, :128],
                                lhsT=cw[f"nexW1_{p}_{gi}"][:],
                                rhs=at[p][:gp, gi, :],
                                start=(p == 0), stop=(p == c.P - 1))
                        h1 = wpool.tile([128, 128], BF, tag="h1")
                        nc.scalar.activation(
                            out=h1[:gp, :], in_=m1[:gp, :128], func=AF.Tanh,
                            bias=cw[f"nexB1_{gi}"][:gp, :])
                        m2 = psM.tile([128, 128], F32, tag="mlp")
                        nc.tensor.matmul(out=m2[:gp, :128],
                                         lhsT=cw[f"nexW2_{gi}"][:],
                                         rhs=h1[:gp, :], start=True, stop=True)
                        nt = wpool.tile([128, 128], BF, tag=f"nt{gi}")
                        nc.scalar.activation(
                            out=nt[:gp, :], in_=m2[:gp, :128], func=AF.Tanh,
                            bias=cw[f"nexB2_{gi}"][:gp, :])
                        nT.append(nt)
                    for p in range(c.P):
                        row = opool.tile([128, c.RB2], BF, tag="row")
                        for half, wkey in ((0, "anW"), (320, "zW")):
                            for gi, g in enumerate(G):
                                gp = 64 * len(g)
                                mm = psM.tile([128, 128], F32, tag="mlp")
                                nc.tensor.matmul(
                                    out=mm[:gp, :128],
                                    lhsT=cw[f"{wkey}_{p}_{gi}"][:],
                                    rhs=nT[gi][:gp, :], start=True, stop=True)
                                sb = wpool.tile([128, 128], BF, tag="anzsb")
                                nc.scalar.activation(out=sb[:gp, :],
                                                     in_=mm[:gp, :128],
                                                     func=AF.Copy)
                                tp = psT.tile([128, 128], BF, tag="tp")
                                nc.tensor.transpose(
                                    out=tp[:, :gp],
                                    in_=sb[:gp, :],
                                    identity=cw["idbf"][:gp, :gp])
                                nc.vector.tensor_copy(
                                    out=row[:, half + 128 * gi:
                                            half + 128 * gi + gp],
                                    in_=tp[:, :gp])
                        nc.vector.memset(row[:, 640:], 0.0)
                        nc.sync.dma_start(
                            out=andz.ap()[p, wq * 128:(wq + 1) * 128, :],
                            in_=row[:])
    nc.compile()
    innames = ([f"xb{p}" for p in range(c.P)]
               + [f"gidx1_{p}_{q}" for p in range(c.P) for q in range(c.NQ)]
               + [f"dloc1_{p}" for p in range(c.P)] + wnames)
    return nc, innames


# ----------------------------------------------------------------------------
# Launch 2 builder (edge + node phase)
# ----------------------------------------------------------------------------

def build_l2(cfg, pr):
    c = cfg
    nc = bacc.Bacc("TRN2", target_bir_lowering=False, debug=False,
                   num_devices=c.NC)

    xbs = [nc.dram_tensor(f"xbs{p}", [c.NKP512, c.RB], BF,
                          kind="ExternalInput") for p in range(c.P)]
    andz = nc.dram_tensor("andz", [c.P, c.HROWS, c.RB2], BF,
                          kind="ExternalInput")
    g2s = [nc.dram_tensor(f"g2s{p}", list(pr.gidx2s[0][p].shape), I16,
                          kind="ExternalInput") for p in range(c.P)]
    g2d = [[nc.dram_tensor(f"g2d{p}_{h}", list(pr.gidx2d[0][p][h].shape), I16,
                           kind="ExternalInput") for h in range(c.NH)]
           for p in range(c.P)]
    sloc = [nc.dram_tensor(f"sloc{p}", list(pr.sloc2[0][p].shape), F32,
                           kind="ExternalInput") for p in range(c.P)]
    icnt = [nc.dram_tensor(f"icnt{p}", [128, c.NB], F32,
                           kind="ExternalInput") for p in range(c.P)]

    wnames = ["iota", "idbf"]
    for p in range(c.P):
        wnames += [f"w2rep_{p}", f"b2rep_{p}"]
        for gi in range(len(c.GRP)):
            wnames += [f"axW_{p}_{gi}", f"axnW_{p}_{gi}", f"u2W_{p}_{gi}",
                       f"axB_{p}_{gi}", f"axnB_{p}_{gi}", f"u2B_{p}_{gi}"]
    wt = {n: nc.dram_tensor(n, list(pr.W[n].shape), _wdt(pr.W[n]),
                            kind="ExternalInput") for n in wnames}

    out = nc.dram_tensor("out", [c.P, c.NKP, 320], F32, kind="ExternalOutput")

    G = c.GRP
    with tile.TileContext(nc) as tc:
        with tc.tile_pool(name="const", bufs=1) as cpool, \
             tc.tile_pool(name="dram", bufs=1, space="DRAM") as dpool, \
             tc.tile_pool(name="gx", bufs=2) as gpool, \
             tc.tile_pool(name="work", bufs=3) as wpool, \
             tc.tile_pool(name="outp", bufs=3) as opool, \
             tc.tile_pool(name="psA", bufs=2, space="PSUM") as psA, \
             tc.tile_pool(name="psM", bufs=2, space="PSUM") as psM, \
             tc.tile_pool(name="psT", bufs=2, space="PSUM") as psT:

            cw = {}
            for n in wnames:
                t = cpool.tile(list(pr.W[n].shape), _wdt(pr.W[n]), tag=n)
                nc.sync.dma_start(out=t[:], in_=wt[n].ap())
                cw[n] = t
            cs = {}
            for p in range(c.P):
                t = cpool.tile(list(pr.gidx2s[0][p].shape), I16, tag=f"g2s{p}")
                nc.sync.dma_start(out=t[:], in_=g2s[p].ap())
                cs[("s", p)] = t
                for h in range(c.NH):
                    t = cpool.tile(list(pr.gidx2d[0][p][h].shape), I16,
                                   tag=f"g2d{p}{h}")
                    nc.sync.dma_start(out=t[:], in_=g2d[p][h].ap())
                    cs[("d", p, h)] = t
                t = cpool.tile(list(pr.sloc2[0][p].shape), F32, tag=f"sl{p}")
                nc.sync.dma_start(out=t[:], in_=sloc[p].ap())
                cs[("l", p)] = t
                t = cpool.tile([128, c.NB], F32, tag=f"ic{p}")
                nc.sync.dma_start(out=t[:], in_=icnt[p].ap())
                cs[("i", p)] = t

            # ---- prologue: ax / axn tables per plane ----
            ax_t = [dpool.tile([c.NKP512, c.RB], BF, tag=f"axT{p}", name=f"axT{p}")
                    for p in range(c.P)]
            axn_t = [dpool.tile([c.NKP512, 320], BF, tag=f"axnT{p}", name=f"axnT{p}")
                     for p in range(c.P)]
            for p in range(c.P):
                for nt_i in range(c.NT512):
                    r0 = nt_i * 512
                    xt = []
                    for j in range(3):
                        xtj = wpool.tile([128, 512], BF, tag=f"xt{j}")
                        nc.sync.dma_start(
                            out=xtj[:],
                            in_=xbs[p].ap()[r0:r0 + 512,
                                            128 * j:128 * (j + 1)],
                            transpose=True)
                        xt.append(xtj)
                    for kind, wkey, bkey, table, ncols in (
                            (0, "axW", "axB", ax_t[p], c.RB),
                            (1, "axnW", "axnB", axn_t[p], 320)):
                        rowt = opool.tile([128, 4, ncols], BF,
                                          tag=f"prow{kind}")
                        for gi, g in enumerate(G):
                            gp = 64 * len(g)
                            mm = psM.tile([128, 512], F32, tag="mlp")
                            nc.tensor.matmul(
                                out=mm[:gp, :],
                                lhsT=cw[f"{wkey}_{p}_{gi}"][:],
                                rhs=xt[gi][:gp, :],
                                start=True, stop=True)
                            sb = wpool.tile([128, 512], BF, tag="presb")
                            nc.scalar.activation(
                                out=sb[:gp, :], in_=mm[:gp, :],
                                func=AF.Identity,
                                bias=cw[f"{bkey}_{p}_{gi}"][:gp, :])
                            for jj in range(4):
                                tp = psT.tile([128, 128], BF, tag="tp")
                                nc.tensor.transpose(
                                    out=tp[:, :gp],
                                    in_=sb[:gp, 128 * jj:128 * (jj + 1)],
                                    identity=cw["idbf"][:gp, :gp])
                                nc.vector.tensor_copy(
                                    out=rowt[:, jj, 128 * gi:128 * gi + gp],
                                    in_=tp[:, :gp])
                        if ncols > 320:
                            nc.vector.memset(rowt[:, :, 320:], 0.0)
                        nc.sync.dma_start(
                            out=table[r0:r0 + 512, :ncols]
                                .rearrange("(a p) d -> p a d", p=128),
                            in_=rowt[:])

            # ---- main loop ----
            maxsegs = max(max((s[1] for p in range(c.P)
                               for s in pr.seg2s[p]), default=1), 1)
            maxsegd = max(max((pr.seg2d[p][h][g][1] for p in range(c.P)
                               for h in range(c.NH)
                               for g in range(pr.NBG)), default=1), 1)
            for p in range(c.P):
                for g in range(pr.NBG):
                    b0, b1 = g * c.BG, min((g + 1) * c.BG, c.NB)
                    st0, sn = pr.seg2s[p][g]
                    gs = gpool.tile([128, maxsegs, c.RB], BF, tag="gs")
                    if sn:
                        emit_gather(nc, gs, ax_t[p][:], cs[("s", p)],
                                    st0, sn, c.RB)
                    gd = {}
                    for h in range(c.NH):
                        dt0, dn = pr.seg2d[p][h][g]
                        gdh = gpool.tile([128, maxsegd, c.RB2], BF,
                                         tag=f"gd{h}")
                        if dn:
                            emit_gather(
                                nc, gdh,
                                andz.ap()[p,
                                          h * c.DCH:
                                          min((h + 1) * c.DCH, c.HROWS), :],
                                cs[("d", p, h)], dt0, dn, c.RB2)
                        gd[h] = (gdh, dt0)
                    axn_g = wpool.tile([128, c.BG, 320], BF, tag="axng")
                    nc.sync.dma_start(
                        out=axn_g[:, :b1 - b0, :],
                        in_=axn_t[p][b0 * 128:b1 * 128, :]
                            .rearrange("(a p) d -> p a d", p=128))

                    for b in range(b0, b1):
                        tl = [(i, h, posh) for i, (b_, h, posh)
                              in enumerate(pr.tiles2[p]) if b_ == b]
                        agg = psA.tile([128, 320], F32, tag="agg")
                        for j, (gti, h, posh) in enumerate(tl):
                            gdh, dt0 = gd[h]
                            axg = gs[:, gti - st0, :]
                            adz = gdh[:, posh - dt0, :]
                            hs = wpool.tile([128, 320], BF, tag="hs")
                            nc.vector.tensor_tensor(
                                out=hs[:], in0=axg[:, :320],
                                in1=adz[:, :320], op=OP.add)
                            ht = wpool.tile([128, 320], BF, tag="ht")
                            nc.scalar.activation(out=ht[:], in_=hs[:],
                                                 func=AF.Tanh)
                            lm = wpool.tile([128, 320], BF, tag="lm")
                            nc.vector.tensor_tensor(
                                out=lm[:], in0=ht[:],
                                in1=cw[f"w2rep_{p}"][:], op=OP.mult)
                            lg = wpool.tile([128, c.C], F32, tag="lg")
                            nc.vector.tensor_reduce(
                                out=lg[:],
                                in_=lm[:].rearrange("q (c f) -> q c f", f=64),
                                axis=mybir.AxisListType.X, op=OP.add)
                            lgb = wpool.tile([128, c.C], F32, tag="lgb")
                            nc.vector.tensor_tensor(
                                out=lgb[:], in0=lg[:],
                                in1=cw[f"b2rep_{p}"][:], op=OP.add)
                            ex = wpool.tile([128, c.C], F32, tag="ex")
                            den = wpool.tile([128, 1], F32, tag="den")
                            nc.scalar.activation(out=ex[:], in_=lgb[:],
                                                 func=AF.Exp,
                                                 accum_out=den[:])
                            rec = wpool.tile([128, 1], F32, tag="rec")
                            nc.vector.reciprocal(out=rec[:], in_=den[:])
                            ws = wpool.tile([128, c.C], F32, tag="ws")
                            nc.scalar.activation(out=ws[:], in_=ex[:],
                                                 func=AF.Copy,
                                                 scale=rec[:, :1])
                            msg = wpool.tile([128, 320], BF, tag="msg")
                            nc.vector.tensor_tensor(
                                out=msg[:].rearrange("q (c f) -> q c f", f=64),
                                in0=adz[:, 320:640]
                                    .rearrange("q (c f) -> q c f", f=64),
                                in1=ws[:].to_broadcast([128, c.C, 64]),
                                op=OP.mult)
                            oh = wpool.tile([128, 128], BF, tag="oh")
                            nc.vector.tensor_tensor(
                                out=oh[:],
                                in0=cs[("l", p)][:, gti:gti + 1]
                                    .to_broadcast([128, 128]),
                                in1=cw["iota"][:], op=OP.is_equal)
                            nc.tensor.matmul(out=agg[:], lhsT=oh[:],
                                             rhs=msg[:],
                                             start=(j == 0),
                                             stop=(j == len(tl) - 1))
                        u1p = wpool.tile([128, 320], F32, tag="u1p")
                        if tl:
                            nc.vector.tensor_scalar_mul(
                                out=u1p[:], in0=agg[:],
                                scalar1=cs[("i", p)][:, b:b + 1])
                        else:
                            nc.vector.memset(u1p[:], 0.0)
                        u1s = wpool.tile([128, 320], F32, tag="u1s")
                        nc.vector.tensor_tensor(
                            out=u1s[:], in0=u1p[:],
                            in1=axn_g[:, b - b0, :], op=OP.add)
                        u1 = wpool.tile([128, 320], BF, tag="u1")
                        nc.scalar.activation(out=u1[:], in_=u1s[:],
                                             func=AF.Tanh)
                        u1t = wpool.tile([128, 3, 128], BF, tag="u1t")
                        for j in range(3):
                            colsj = 320 - 128 * j if j == 2 else 128
                            tp = psT.tile([128, 128], BF, tag="tp")
                            nc.tensor.transpose(
                                out=tp[:colsj, :],
                                in_=u1[:, 128 * j:128 * j + colsj],
                                identity=cw["idbf"][:])
                            nc.vector.tensor_copy(out=u1t[:colsj, j, :],
                                                  in_=tp[:colsj, :])
                        orow = opool.tile([128, 320], F32, tag="orow")
                        for gi, gcl in enumerate(G):
                            gp = 64 * len(gcl)
                            mm = psM.tile([128, 512], F32, tag="mlp")
                            nc.tensor.matmul(out=mm[:gp, :128],
                                             lhsT=cw[f"u2W_{p}_{gi}"][:],
                                             rhs=u1t[:gp, gi, :],
                                             start=True, stop=True)
                            u2 = wpool.tile([128, 128], BF, tag="u2")
                            nc.scalar.activation(
                                out=u2[:gp, :], in_=mm[:gp, :128],
                                func=AF.Tanh,
                                bias=cw[f"u2B_{p}_{gi}"][:gp, :])
                            tp = psT.tile([128, 128], BF, tag="tp")
                            nc.tensor.transpose(out=tp[:, :gp],
                                                in_=u2[:gp, :],
                                                identity=cw["idbf"][:gp, :gp])
                            nc.vector.tensor_copy(
                                out=orow[:, 128 * gi:128 * gi + gp],
                                in_=tp[:, :gp])
                        nc.sync.dma_start(
                            out=out.ap()[p, b * 128:(b + 1) * 128, :],
                            in_=orow[:])
    nc.compile()
    innames = ([f"xbs{p}" for p in range(c.P)] + ["andz"]
               + [f"g2s{p}" for p in range(c.P)]
               + [f"g2d{p}_{h}" for p in range(c.P) for h in range(c.NH)]
               + [f"sloc{p}" for p in range(c.P)]
               + [f"icnt{p}" for p in range(c.P)] + wnames)
    return nc, innames


# ----------------------------------------------------------------------------
# in_maps
# ----------------------------------------------------------------------------

def l1_inmaps(cfg, pr, names):
    c = cfg
    maps = []
    for k in range(c.NC):
        m = {}
        for p in range(c.P):
            m[f"xb{p}"] = pr.xb[p]
            for q in range(c.NQ):
                m[f"gidx1_{p}_{q}"] = pr.gidx1[k][p][q]
            m[f"dloc1_{p}"] = pr.dloc1[k][p]
        for n in pr.W:
            m[n] = pr.W[n]
        maps.append({n: m[n] for n in names})
    return maps


def l2_inmaps(cfg, pr, andz_full, names):
    c = cfg
    maps = []
    for k in range(c.NC):
        m = {"andz": andz_full}
        for p in range(c.P):
            m[f"xbs{p}"] = pr.xb_slice[k][p]
            m[f"g2s{p}"] = pr.gidx2s[k][p]
            for h in range(c.NH):
                m[f"g2d{p}_{h}"] = pr.gidx2d[k][p][h]
            m[f"sloc{p}"] = pr.sloc2[k][p]
            m[f"icnt{p}"] = pr.invcnt[k][p]
        for n in pr.W:
            m[n] = pr.W[n]
        maps.append({n: m[n] for n in names})
    return maps


# ----------------------------------------------------------------------------
# public kernel()
# ----------------------------------------------------------------------------

def run(cfg, inputs, runner=None):
    """runner(nc, maps) -> list of per-core output dicts; default = HW SPMD."""
    pr = host_prep(cfg, inputs)

    nc1, in1 = build_l1(cfg, pr)
    maps1 = l1_inmaps(cfg, pr, in1)
    if runner is None:
        res1 = run_bass_kernel_spmd(nc1, maps1,
                                    core_ids=list(range(cfg.NC))).results
    else:
        res1 = runner(nc1, maps1)
    andz_full = np.concatenate([res1[k]["andz"] for k in range(cfg.NC)],
                               axis=1)

    nc2, in2 = build_l2(cfg, pr)
    maps2 = l2_inmaps(cfg, pr, andz_full, in2)
    if runner is None:
        res2 = run_bass_kernel_spmd(nc2, maps2,
                                    core_ids=list(range(cfg.NC))).results
    else:
        res2 = runner(nc2, maps2)

    out = np.concatenate([res2[k]["out"][:, :cfg.NK, :]
                          for k in range(cfg.NC)], axis=1)
    return np.ascontiguousarray(
        out.reshape(cfg.P, cfg.N, cfg.C, cfg.F).astype(np.float32))


def kernel(**inputs):
    return run(Cfg(), inputs)


# revision 14
# speedup vs baseline: 44.7738x; 44.7738x over previous
"""NexusNet Trainium2 kernel (8-core SPMD, Bass/Tile).

Self-contained: only imports numpy + jax (for bf16 cast) + concourse.

Strategy (graph/data parallel per the sharding hint): two SPMD launches.
  L1 (dst-partition): each core owns M/8 nexus rows. Per-edge dma_gather of
     x[src] rows (bf16), one-hot segment-sum matmuls into PSUM windows,
     nexus MLP, then per-nexus-row linear precomputes:
        an = W1n_edge^T n   (edge-MLP n-side term, per plane)
        z  = W1n_node^T n   (node-MLP n-side term, per plane)
     written as packed rows [an | z] (768 bf16 = 1536B) per plane.
  Host concatenates the 8 andz slices (index-space stitch only).
  L2 (src-partition): each core owns N/8 plane nodes + their edges.
     Prologue per plane: ax = x W1x_edge + b1 (per node), axn = x W1x_node + b1n.
     Per edge tile: gather ax[src], [an|z][dst]; h = tanh(ax+an);
     logits = reduce(h*w2)+b2; softmax over classes; msg = w * z;
     weighted one-hot segment matmul -> agg per 128-node block;
     u1 = tanh(axn + invcnt*agg); u2 = tanh(u1 W2 + b2n) -> output.
"""

import numpy as np

from concourse import bacc, bass, mybir, tile
from concourse.bass_utils import run_bass_kernel_spmd

DT = mybir.dt
BF = DT.bfloat16
F32 = DT.float32
I16 = DT.int16
AF = mybir.ActivationFunctionType
OP = mybir.AluOpType


def ceil_div(a, b):
    return (a + b - 1) // b


class Cfg:
    def __init__(self, N=80000, M=40000, E=120000, ncores=8,
                 chunk=20000, nwh=20, bg=8, dch=None):
        self.N, self.M, self.E = N, M, E
        self.C, self.F, self.S, self.EF, self.P = 5, 64, 64, 64, 3
        self.RB = 384          # padded bf16 row elems (C*F=320 -> 384; 768B)
        self.RB2 = 768         # [an|z] row elems (1536B)
        self.NC = ncores
        assert N % ncores == 0 and M % ncores == 0
        self.NK = N // ncores              # plane nodes per core
        self.MK = M // ncores              # nexus rows per core
        self.NW = ceil_div(self.MK, 128)   # L1 windows per core
        self.MKP = self.NW * 128           # padded andz rows per core
        self.NB = ceil_div(self.NK, 128)   # L2 node blocks per core
        self.NKP = self.NB * 128
        self.NT512 = ceil_div(self.NKP, 512)
        self.NKP512 = self.NT512 * 512
        self.CHUNK = min(chunk, N)         # L1 src chunk rows (int16 limit)
        self.NQ = ceil_div(N, self.CHUNK)
        self.HROWS = ncores * self.MKP     # stitched andz rows
        self.DCH = dch if dch is not None else (
            self.HROWS if self.HROWS <= 32767 else ceil_div(self.HROWS, 2))
        self.NH = ceil_div(self.HROWS, self.DCH)
        assert self.CHUNK <= 32767 and self.DCH <= 32767 and self.NK <= 32767
        self.NWH = min(nwh, self.NW)       # windows per L1 half-pass
        self.NHALF = ceil_div(self.NW, self.NWH)
        self.BG = bg                       # L2 blocks per gather group
        self.GRP = [(0, 1), (2, 3), (4,)]  # class pair groups


def wrap_idx(idx):
    """[n] int array -> [128, n//16] int16 wrapped+replicated layout."""
    n = len(idx)
    assert n % 16 == 0
    w16 = np.asarray(idx, np.int16).reshape(n // 16, 16).T.copy()
    return np.tile(w16, (8, 1))


def blkdiag(mats):
    rs = sum(m.shape[0] for m in mats)
    cs = sum(m.shape[1] for m in mats)
    out = np.zeros((rs, cs), np.float32)
    r = c = 0
    for m in mats:
        out[r:r + m.shape[0], c:c + m.shape[1]] = m
        r += m.shape[0]
        c += m.shape[1]
    return out


def to_bf16(x):
    import ml_dtypes
    return np.asarray(x).astype(ml_dtypes.bfloat16)


def pack_rows_bf16(x2d, rb):
    n, d = x2d.shape
    out = np.zeros((n, rb), dtype=to_bf16(np.zeros(1)).dtype)
    out[:, :d] = to_bf16(x2d)
    return out


# ----------------------------------------------------------------------------
# Host preprocessing
# ----------------------------------------------------------------------------

class Prep:
    pass


def host_prep(cfg, inputs):
    c = cfg
    pr = Prep()
    planes = "uvy"

    xs = [np.asarray(inputs[f"x_{p}"], np.float32).reshape(c.N, c.C * c.F)
          for p in planes]
    edges = [np.asarray(inputs[f"edge_{p}"], np.int64) for p in planes]

    pr.xb = [pack_rows_bf16(x, c.RB) for x in xs]
    pr.xb_slice = [[None] * c.P for _ in range(c.NC)]
    for k in range(c.NC):
        for p in range(c.P):
            s = np.zeros((c.NKP512, c.RB), pr.xb[0].dtype)
            s[:c.NK] = pr.xb[p][k * c.NK:(k + 1) * c.NK]
            pr.xb_slice[k][p] = s

    # ---------------- L1 edge grouping: (plane, window, chunk) ----------
    l1_lists = [[[[None] * c.NQ for _ in range(c.NW)] for _ in range(c.P)]
                for _ in range(c.NC)]
    for p in range(c.P):
        src, dst = edges[p][0], edges[p][1]
        owner = dst // c.MK
        for k in range(c.NC):
            sel = owner == k
            s = src[sel]
            d = dst[sel] - k * c.MK
            wi = d // 128
            q = s // c.CHUNK
            for w in range(c.NW):
                for qq in range(c.NQ):
                    m = (wi == w) & (q == qq)
                    l1_lists[k][p][w][qq] = (s[m] - qq * c.CHUNK, d[m] - w * 128)

    pr.T1 = [[[0] * c.NQ for _ in range(c.NW)] for _ in range(c.P)]
    for p in range(c.P):
        for w in range(c.NW):
            for q in range(c.NQ):
                mx = max(len(l1_lists[k][p][w][q][0]) for k in range(c.NC))
                pr.T1[p][w][q] = ceil_div(mx, 128)

    # global tile order: wi asc, q asc, t ; chunk order: wi asc, t
    pr.tiles1 = []          # per p: list of (wi, q, pos_in_q)
    pr.qtiles1 = []
    for p in range(c.P):
        qpos = [0] * c.NQ
        tl = []
        for w in range(c.NW):
            for q in range(c.NQ):
                for t in range(pr.T1[p][w][q]):
                    tl.append((w, q, qpos[q]))
                    qpos[q] += 1
        pr.tiles1.append(tl)
        pr.qtiles1.append(qpos)

    pr.seg1 = [[[None] * c.NHALF for _ in range(c.NQ)] for _ in range(c.P)]
    for p in range(c.P):
        for q in range(c.NQ):
            pos = 0
            for hf in range(c.NHALF):
                w0, w1 = hf * c.NWH, min((hf + 1) * c.NWH, c.NW)
                n = sum(pr.T1[p][w][q] for w in range(w0, w1))
                pr.seg1[p][q][hf] = (pos, n)
                pos += n

    pr.gidx1 = [[[None] * c.NQ for _ in range(c.P)] for _ in range(c.NC)]
    pr.dloc1 = [[None] * c.P for _ in range(c.NC)]
    for k in range(c.NC):
        for p in range(c.P):
            ntile_tot = len(pr.tiles1[p])
            dl = np.full((128, max(ntile_tot, 1)), -1.0, np.float32)
            per_q = [np.zeros(max(pr.qtiles1[p][q], 1) * 128, np.int64)
                     for q in range(c.NQ)]
            consumed = {}
            for gti, (w, q, posq) in enumerate(pr.tiles1[p]):
                s_arr, d_arr = l1_lists[k][p][w][q]
                off = consumed.get((w, q), 0)
                take = s_arr[off:off + 128]
                dtake = d_arr[off:off + 128]
                consumed[(w, q)] = off + 128
                nslot = len(take)
                per_q[q][posq * 128:posq * 128 + nslot] = take
                dl[:nslot, gti] = dtake
            for q in range(c.NQ):
                pr.gidx1[k][p][q] = wrap_idx(per_q[q])
            pr.dloc1[k][p] = dl

    # ---------------- L2 edge grouping: (plane, block, hchunk) ----------
    l2_lists = [[[[None] * c.NH for _ in range(c.NB)] for _ in range(c.P)]
                for _ in range(c.NC)]
    pr.invcnt = [[None] * c.P for _ in range(c.NC)]
    for p in range(c.P):
        src, dst = edges[p][0], edges[p][1]
        owner = src // c.NK
        drow = (dst // c.MK) * c.MKP + (dst % c.MK)
        for k in range(c.NC):
            sel = owner == k
            s = src[sel] - k * c.NK
            dr = drow[sel]
            b = s // 128
            h = dr // c.DCH
            for bb in range(c.NB):
                for hh in range(c.NH):
                    m = (b == bb) & (h == hh)
                    l2_lists[k][p][bb][hh] = (s[m], dr[m] - hh * c.DCH,
                                              s[m] - bb * 128)
            deg = np.bincount(s, minlength=c.NKP).astype(np.float32)
            ic = 1.0 / np.maximum(deg, 1.0)
            pr.invcnt[k][p] = ic[:c.NKP].reshape(c.NB, 128).T.copy()

    pr.T2 = [[[0] * c.NH for _ in range(c.NB)] for _ in range(c.P)]
    for p in range(c.P):
        for b in range(c.NB):
            for h in range(c.NH):
                mx = max(len(l2_lists[k][p][b][h][0]) for k in range(c.NC))
                pr.T2[p][b][h] = ceil_div(mx, 128)

    pr.tiles2 = []          # per p: list of (b, h, pos_in_h)
    pr.htiles2 = []
    for p in range(c.P):
        hpos = [0] * c.NH
        tl = []
        for b in range(c.NB):
            for h in range(c.NH):
                for t in range(pr.T2[p][b][h]):
                    tl.append((b, h, hpos[h]))
                    hpos[h] += 1
        pr.tiles2.append(tl)
        pr.htiles2.append(hpos)

    pr.NBG = ceil_div(c.NB, c.BG)
    pr.seg2s = [[None] * pr.NBG for _ in range(c.P)]
    pr.seg2d = [[[None] * pr.NBG for _ in range(c.NH)] for _ in range(c.P)]
    for p in range(c.P):
        gpos = 0
        hseen = [0] * c.NH
        for g in range(pr.NBG):
            b0, b1 = g * c.BG, min((g + 1) * c.BG, c.NB)
            n = sum(pr.T2[p][b][h] for b in range(b0, b1) for h in range(c.NH))
            pr.seg2s[p][g] = (gpos, n)
            gpos += n
            for h in range(c.NH):
                nh = sum(pr.T2[p][b][h] for b in range(b0, b1))
                pr.seg2d[p][h][g] = (hseen[h], nh)
                hseen[h] += nh

    pr.gidx2s = [[None] * c.P for _ in range(c.NC)]
    pr.gidx2d = [[[None] * c.NH for _ in range(c.P)] for _ in range(c.NC)]
    pr.sloc2 = [[None] * c.P for _ in range(c.NC)]
    for k in range(c.NC):
        for p in range(c.P):
            ntile_tot = len(pr.tiles2[p])
            sl = np.full((128, max(ntile_tot, 1)), -1.0, np.float32)
            sg = np.zeros(max(ntile_tot, 1) * 128, np.int64)
            per_h = [np.zeros(max(pr.htiles2[p][h], 1) * 128, np.int64)
                     for h in range(c.NH)]
            consumed = {}
            for gti, (b, h, posh) in enumerate(pr.tiles2[p]):
                s_arr, d_arr, loc_arr = l2_lists[k][p][b][h]
                off = consumed.get((b, h), 0)
                take_s = s_arr[off:off + 128]
                take_d = d_arr[off:off + 128]
                take_l = loc_arr[off:off + 128]
                consumed[(b, h)] = off + 128
                nslot = len(take_s)
                sg[gti * 128:gti * 128 + nslot] = take_s
                per_h[h][posh * 128:posh * 128 + nslot] = take_d
                sl[:nslot, gti] = take_l
            pr.gidx2s[k][p] = wrap_idx(sg)
            for h in range(c.NH):
                pr.gidx2d[k][p][h] = wrap_idx(per_h[h])
            pr.sloc2[k][p] = sl

    # ---------------- weights packing ------------------------------------
    C, F = c.C, c.F
    nex_w1 = np.asarray(inputs["nex_w1"], np.float32)
    nex_b1 = np.asarray(inputs["nex_b1"], np.float32)
    nex_w2 = np.asarray(inputs["nex_w2"], np.float32)
    nex_b2 = np.asarray(inputs["nex_b2"], np.float32)
    edge_w1 = np.asarray(inputs["edge_w1"], np.float32)
    edge_b1 = np.asarray(inputs["edge_b1"], np.float32)
    edge_w2 = np.asarray(inputs["edge_w2"], np.float32)
    edge_b2 = np.asarray(inputs["edge_b2"], np.float32)
    node_w1 = np.asarray(inputs["node_w1"], np.float32)
    node_b1 = np.asarray(inputs["node_b1"], np.float32)
    node_w2 = np.asarray(inputs["node_w2"], np.float32)
    node_b2 = np.asarray(inputs["node_b2"], np.float32)

    G = c.GRP
    W = {}
    for gi, g in enumerate(G):
        for p in range(c.P):
            W[f"nexW1_{p}_{gi}"] = to_bf16(
                blkdiag([nex_w1[cc, p * F:(p + 1) * F, :] for cc in g]))
            W[f"anW_{p}_{gi}"] = to_bf16(
                blkdiag([edge_w1[p, cc, F:, :] for cc in g]))
            W[f"zW_{p}_{gi}"] = to_bf16(
                blkdiag([node_w1[p, cc, F:, :] for cc in g]))
            W[f"axW_{p}_{gi}"] = to_bf16(
                blkdiag([edge_w1[p, cc, :F, :] for cc in g]))
            W[f"axnW_{p}_{gi}"] = to_bf16(
                blkdiag([node_w1[p, cc, :F, :] for cc in g]))
            W[f"u2W_{p}_{gi}"] = to_bf16(
                blkdiag([node_w2[p, cc] for cc in g]))
            W[f"axB_{p}_{gi}"] = np.concatenate(
                [edge_b1[p, cc] for cc in g])[:, None].astype(np.float32)
            W[f"axnB_{p}_{gi}"] = np.concatenate(
                [node_b1[p, cc] for cc in g])[:, None].astype(np.float32)
            W[f"u2B_{p}_{gi}"] = np.concatenate(
                [node_b2[p, cc] for cc in g])[:, None].astype(np.float32)
        W[f"nexW2_{gi}"] = to_bf16(blkdiag([nex_w2[cc] for cc in g]))
        W[f"nexB1_{gi}"] = np.concatenate(
            [nex_b1[cc] for cc in g])[:, None].astype(np.float32)
        W[f"nexB2_{gi}"] = np.concatenate(
            [nex_b2[cc] for cc in g])[:, None].astype(np.float32)
    for p in range(c.P):
        W[f"w2rep_{p}"] = np.tile(
            to_bf16(edge_w2[p, :, :, 0].reshape(1, C * c.EF)), (128, 1))
        W[f"b2rep_{p}"] = np.tile(edge_b2[p, :, 0].reshape(1, C),
                                  (128, 1)).astype(np.float32)
    W["iota"] = np.tile(np.arange(128, dtype=np.float32), (128, 1))
    W["idbf"] = to_bf16(np.eye(128, dtype=np.float32))
    W["idf32"] = np.eye(128, dtype=np.float32)
    pr.W = W
    return pr


def _wdt(arr):
    return BF if arr.dtype != np.float32 else F32


GATHER_MAX_TILES = 8   # 1024 idxs: SWDGE descriptor ring holds only 1024


def emit_gather2(nc, pool, tag, in_ap, idx_tile, t0, ntiles, elem, dt=BF):
    """dma_gather of ntiles*128 rows via ring-sized sub-calls, each into its
    own pool tile (deep pipelining across SDMA engines).

    Returns `sub(j) -> AP` mapping seg-local tile j to its [128, elem] slice.
    """
    subs = []
    done = 0
    while done < ntiles:
        n = min(GATHER_MAX_TILES, ntiles - done)
        gt = pool.tile([128, GATHER_MAX_TILES, elem], dt, tag=tag,
                       name=f"{tag}_{t0}_{done}")
        nc.gpsimd.dma_gather(
            out_ap=gt[:, :n, :],
            in_ap=in_ap,
            idxs_ap=idx_tile[:, (t0 + done) * 8:(t0 + done + n) * 8],
            num_idxs=n * 128,
            num_idxs_reg=n * 128,
            elem_size=elem,
        )
        subs.append(gt)
        done += n

    def sub(j):
        return subs[j // GATHER_MAX_TILES][:, j % GATHER_MAX_TILES, :]
    return sub


# ----------------------------------------------------------------------------
# Launch 1 builder (nexus phase)
# ----------------------------------------------------------------------------

def build_l1(cfg, pr, mode="full"):
    c = cfg
    nc = bacc.Bacc("TRN2", target_bir_lowering=False, debug=False,
                   num_devices=c.NC)

    xb = [nc.dram_tensor(f"xb{p}", [c.N, c.RB], BF, kind="ExternalInput")
          for p in range(c.P)]
    gidx = [[nc.dram_tensor(f"gidx1_{p}_{q}",
                            list(pr.gidx1[0][p][q].shape), I16,
                            kind="ExternalInput")
             for q in range(c.NQ)] for p in range(c.P)]
    dloc = [nc.dram_tensor(f"dloc1_{p}", list(pr.dloc1[0][p].shape), F32,
                           kind="ExternalInput") for p in range(c.P)]

    wnames = ["iota", "idbf", "idf32"]
    for gi in range(len(c.GRP)):
        wnames += [f"nexW2_{gi}", f"nexB1_{gi}", f"nexB2_{gi}"]
        for p in range(c.P):
            wnames += [f"nexW1_{p}_{gi}", f"anW_{p}_{gi}", f"zW_{p}_{gi}"]
    wt = {n: nc.dram_tensor(n, list(pr.W[n].shape), _wdt(pr.W[n]),
                            kind="ExternalInput") for n in wnames}

    andz = nc.dram_tensor("andz", [c.P, c.MKP, c.RB2], BF,
                          kind="ExternalOutput")

    G = c.GRP
    with tile.TileContext(nc) as tc:
        with tc.tile_pool(name="const", bufs=1) as cpool, \
             tc.tile_pool(name="asb", bufs=1) as apool, \
             tc.tile_pool(name="gx", bufs=6) as gpool, \
             tc.tile_pool(name="work", bufs=3) as wpool, \
             tc.tile_pool(name="outp", bufs=3) as opool, \
             tc.tile_pool(name="psA", bufs=2, space="PSUM") as psA, \
             tc.tile_pool(name="psM", bufs=2, space="PSUM") as psM, \
             tc.tile_pool(name="psT", bufs=2, space="PSUM") as psT:

            cw = {}
            for n in wnames:
                t = cpool.tile(list(pr.W[n].shape), _wdt(pr.W[n]), tag=n)
                nc.sync.dma_start(out=t[:], in_=wt[n].ap())
                cw[n] = t
            cidx = {}
            for p in range(c.P):
                for q in range(c.NQ):
                    t = cpool.tile(list(pr.gidx1[0][p][q].shape), I16,
                                   tag=f"gi{p}_{q}")
                    nc.sync.dma_start(out=t[:], in_=gidx[p][q].ap())
                    cidx[(p, q)] = t
            cdl = {}
            for p in range(c.P):
                t = cpool.tile(list(pr.dloc1[0][p].shape), F32, tag=f"dl{p}")
                nc.sync.dma_start(out=t[:], in_=dloc[p].ap())
                cdl[p] = t

            A = [apool.tile([128, c.NWH, 320], F32, tag=f"A{p}", name=f"A{p}")
                 for p in range(c.P)]

            maxseg = max(max((pr.seg1[p][q][hf][1] for q in range(c.NQ)
                              for hf in range(c.NHALF)), default=1), 1)

            for hf in range(c.NHALF):
                w0 = hf * c.NWH
                w1 = min(w0 + c.NWH, c.NW)
                # ---- phase A: gathers + window accumulation ----
                for p in range(c.P):
                    for q in range(c.NQ):
                        t0, nseg = pr.seg1[p][q][hf]
                        if nseg == 0:
                            continue
                        gsub = emit_gather2(
                            nc, gpool, "gx",
                            xb[p].ap()[q * c.CHUNK:
                                       min((q + 1) * c.CHUNK, c.N), :],
                            cidx[(p, q)], t0, nseg, c.RB)
                        if mode == "gather":
                            continue
                        byw = {}
                        for gti, (wq, q_, posq) in enumerate(pr.tiles1[p]):
                            if q_ == q and w0 <= wq < w1:
                                byw.setdefault(wq, []).append((gti, posq))
                        for wq in sorted(byw):
                            tl = byw[wq]
                            aps = psA.tile([128, 320], F32, tag="Aps")
                            for j, (gti, posq) in enumerate(tl):
                                oh = wpool.tile([128, 128], BF, tag="oh")
                                nc.vector.tensor_tensor(
                                    out=oh[:],
                                    in0=cdl[p][:, gti:gti + 1]
                                        .to_broadcast([128, 128]),
                                    in1=cw["iota"][:],
                                    op=OP.is_equal)
                                nc.tensor.matmul(
                                    out=aps[:],
                                    lhsT=oh[:],
                                    rhs=gsub(posq - t0)[:, :320],
                                    start=(j == 0), stop=(j == len(tl) - 1))
                            dst = A[p][:, wq - w0, :]
                            first = all(pr.T1[p][wq][qq] == 0
                                        for qq in range(q))
                            if first:
                                nc.vector.tensor_copy(out=dst, in_=aps[:])
                            else:
                                nc.vector.tensor_tensor(
                                    out=dst, in0=dst, in1=aps[:], op=OP.add)
                for p in range(c.P):
                    for wq in range(w0, w1):
                        if mode == "full" and all(
                                pr.T1[p][wq][q] == 0 for q in range(c.NQ)):
                            nc.vector.memset(A[p][:, wq - w0, :], 0.0)

                # ---- phase B: per-window MLP ----
                for wq in (range(w0, w1) if mode == "full" else []):
                    wl = wq - w0
                    at = []
                    for p in range(c.P):
                        atp = wpool.tile([128, 3, 128], BF, tag=f"at{p}")
                        for j in range(3):
                            cols = 320 - 128 * j if j == 2 else 128
                            tp = psT.tile([128, 128], F32, tag="tp")
                            nc.tensor.transpose(
                                out=tp[:cols, :128],
                                in_=A[p][:, wl, 128 * j:128 * j + cols],
                                identity=cw["idf32"][:])
                            nc.vector.tensor_copy(out=atp[:cols, j, :],
                                                  in_=tp[:cols, :128])
                        at.append(atp)
                    nT = []
                    for gi, g in enumerate(G):
                        gp = 64 * len(g)
                        m1 = psM.tile([128, 128], F32, tag="mlp")
                        for p in range(c.P):
                            nc.tensor.matmul(
                                out=m1[:gp, :128],
                                lhsT=cw[f"nexW1_{p}_{gi}"][:],
                                rhs=at[p][:gp, gi, :],
                                start=(p == 0), stop=(p == c.P - 1))
                        h1 = wpool.tile([128, 128], BF, tag="h1")
                        nc.scalar.activation(
                            out=h1[:gp, :], in_=m1[:gp, :128], func=AF.Tanh,
                            bias=cw[f"nexB1_{gi}"][:gp, :])
                        m2 = psM.tile([128, 128], F32, tag="mlp")
                        nc.tensor.matmul(out=m2[:gp, :128],
                                         lhsT=cw[f"nexW2_{gi}"][:],
                                         rhs=h1[:gp, :], start=True, stop=True)
                        nt = wpool.tile([128, 128], BF, tag=f"nt{gi}")
                        nc.scalar.activation(
                            out=nt[:gp, :], in_=m2[:gp, :128], func=AF.Tanh,
                            bias=cw[f"nexB2_{gi}"][:gp, :])
                        nT.append(nt)
                    for p in range(c.P):
                        row = opool.tile([128, c.RB2], BF, tag="row")
                        for half, wkey in ((0, "anW"), (320, "zW")):
                            for gi, g in enumerate(G):
                                gp = 64 * len(g)
                                mm = psM.tile([128, 128], F32, tag="mlp")
                                nc.tensor.matmul(
                                    out=mm[:gp, :128],
                                    lhsT=cw[f"{wkey}_{p}_{gi}"][:],
                                    rhs=nT[gi][:gp, :], start=True, stop=True)
                                sb = wpool.tile([128, 128], BF, tag="anzsb")
                                nc.scalar.activation(out=sb[:gp, :],
                                                     in_=mm[:gp, :128],
                                                     func=AF.Copy)
                                tp = psT.tile([128, 128], BF, tag="tp")
                                nc.tensor.transpose(
                                    out=tp[:, :gp],
                                    in_=sb[:gp, :],
                                    identity=cw["idbf"][:gp, :gp])
                                nc.vector.tensor_copy(
                                    out=row[:, half + 128 * gi:
                                            half + 128 * gi + gp],
                                    in_=tp[:, :gp])
                        nc.sync.dma_start(
                            out=andz.ap()[p, wq * 128:(wq + 1) * 128, :640],
                            in_=row[:, :640])
            if mode != "full":
                with tc.tile_pool(name="dummy", bufs=1) as dp:
                    zrow = dp.tile([128, c.RB2], BF)
                    nc.vector.memset(zrow[:], 0.0)
                    nc.sync.dma_start(out=andz.ap()[0, 0:128, :], in_=zrow[:])
    nc.compile()
    innames = ([f"xb{p}" for p in range(c.P)]
               + [f"gidx1_{p}_{q}" for p in range(c.P) for q in range(c.NQ)]
               + [f"dloc1_{p}" for p in range(c.P)] + wnames)
    return nc, innames


# ----------------------------------------------------------------------------
# Launch 2 builder (edge + node phase)
# ----------------------------------------------------------------------------

def build_l2(cfg, pr):
    c = cfg
    nc = bacc.Bacc("TRN2", target_bir_lowering=False, debug=False,
                   num_devices=c.NC)

    xbs = [nc.dram_tensor(f"xbs{p}", [c.NKP512, c.RB], BF,
                          kind="ExternalInput") for p in range(c.P)]
    andz = nc.dram_tensor("andz", [c.P, c.HROWS, c.RB2], BF,
                          kind="ExternalInput")
    g2d = [[nc.dram_tensor(f"g2d{p}_{h}", list(pr.gidx2d[0][p][h].shape), I16,
                           kind="ExternalInput") for h in range(c.NH)]
           for p in range(c.P)]
    sloc = [nc.dram_tensor(f"sloc{p}", list(pr.sloc2[0][p].shape), F32,
                           kind="ExternalInput") for p in range(c.P)]
    icnt = [nc.dram_tensor(f"icnt{p}", [128, c.NB], F32,
                           kind="ExternalInput") for p in range(c.P)]

    wnames = ["iota", "idbf"]
    for p in range(c.P):
        wnames += [f"w2rep_{p}", f"b2rep_{p}"]
        for gi in range(len(c.GRP)):
            wnames += [f"axW_{p}_{gi}", f"axnW_{p}_{gi}", f"u2W_{p}_{gi}",
                       f"axB_{p}_{gi}", f"axnB_{p}_{gi}", f"u2B_{p}_{gi}"]
    wt = {n: nc.dram_tensor(n, list(pr.W[n].shape), _wdt(pr.W[n]),
                            kind="ExternalInput") for n in wnames}

    out = nc.dram_tensor("out", [c.P, c.NKP, 320], F32, kind="ExternalOutput")

    G = c.GRP
    with tile.TileContext(nc) as tc:
        with tc.tile_pool(name="const", bufs=1) as cpool, \
             tc.tile_pool(name="dram", bufs=1, space="DRAM") as dpool, \
             tc.tile_pool(name="gd", bufs=4) as gdpool, \
             tc.tile_pool(name="axx", bufs=3) as axpool, \
             tc.tile_pool(name="work", bufs=3) as wpool, \
             tc.tile_pool(name="outp", bufs=3) as opool, \
             tc.tile_pool(name="psA", bufs=2, space="PSUM") as psA, \
             tc.tile_pool(name="psS", bufs=2, space="PSUM") as psS, \
             tc.tile_pool(name="psM", bufs=2, space="PSUM") as psM, \
             tc.tile_pool(name="psT", bufs=2, space="PSUM") as psT:

            cw = {}
            for n in wnames:
                t = cpool.tile(list(pr.W[n].shape), _wdt(pr.W[n]), tag=n)
                nc.sync.dma_start(out=t[:], in_=wt[n].ap())
                cw[n] = t
            cs = {}
            for p in range(c.P):
                for h in range(c.NH):
                    t = cpool.tile(list(pr.gidx2d[0][p][h].shape), I16,
                                   tag=f"g2d{p}{h}")
                    nc.sync.dma_start(out=t[:], in_=g2d[p][h].ap())
                    cs[("d", p, h)] = t
                t = cpool.tile(list(pr.sloc2[0][p].shape), F32, tag=f"sl{p}")
                nc.sync.dma_start(out=t[:], in_=sloc[p].ap())
                cs[("l", p)] = t
                t = cpool.tile([128, c.NB], F32, tag=f"ic{p}")
                nc.sync.dma_start(out=t[:], in_=icnt[p].ap())
                cs[("i", p)] = t

            # ---- prologue: merged [ax | axn] table per plane ----
            axx_t = [dpool.tile([c.NKP512, 640], BF, tag=f"axxT{p}",
                                name=f"axxT{p}") for p in range(c.P)]
            for p in range(c.P):
                for nt_i in range(c.NT512):
                    r0 = nt_i * 512
                    xt = []
                    for j in range(3):
                        xtj = wpool.tile([128, 512], BF, tag=f"xt{j}")
                        nc.sync.dma_start(
                            out=xtj[:],
                            in_=xbs[p].ap()[r0:r0 + 512,
                                            128 * j:128 * (j + 1)],
                            transpose=True)
                        xt.append(xtj)
                    for coloff, wkey, bkey in ((0, "axW", "axB"),
                                               (320, "axnW", "axnB")):
                        rowt = opool.tile([128, 4, 320], BF,
                                          tag=f"prow{coloff}")
                        for gi, g in enumerate(G):
                            gp = 64 * len(g)
                            mm = psM.tile([128, 512], F32, tag="mlp")
                            nc.tensor.matmul(
                                out=mm[:gp, :],
                                lhsT=cw[f"{wkey}_{p}_{gi}"][:],
                                rhs=xt[gi][:gp, :],
                                start=True, stop=True)
                            sb = wpool.tile([128, 512], BF, tag="presb")
                            nc.scalar.activation(
                                out=sb[:gp, :], in_=mm[:gp, :],
                                func=AF.Identity,
                                bias=cw[f"{bkey}_{p}_{gi}"][:gp, :])
                            for jj in range(4):
                                tp = psT.tile([128, 128], BF, tag="tp")
                                nc.tensor.transpose(
                                    out=tp[:, :gp],
                                    in_=sb[:gp, 128 * jj:128 * (jj + 1)],
                                    identity=cw["idbf"][:gp, :gp])
                                nc.vector.tensor_copy(
                                    out=rowt[:, jj, 128 * gi:128 * gi + gp],
                                    in_=tp[:, :gp])
                        nc.sync.dma_start(
                            out=axx_t[p][r0:r0 + 512, coloff:coloff + 320]
                                .rearrange("(a p) d -> p a d", p=128),
                            in_=rowt[:])

            # ---- main loop ----
            for p in range(c.P):
                for g in range(pr.NBG):
                    b0, b1 = g * c.BG, min((g + 1) * c.BG, c.NB)
                    gd = {}
                    for h in range(c.NH):
                        dt0, dn = pr.seg2d[p][h][g]
                        gd[h] = (emit_gather2(
                            nc, gdpool, f"gd{h}",
                            andz.ap()[p, h * c.DCH:
                                      min((h + 1) * c.DCH, c.HROWS), :],
                            cs[("d", p, h)], dt0, dn, c.RB2)
                            if dn else None, dt0)
                    axx_g = axpool.tile([128, c.BG, 640], BF, tag="axxg")
                    nc.sync.dma_start(
                        out=axx_g[:, :b1 - b0, :],
                        in_=axx_t[p][b0 * 128:b1 * 128, :]
                            .rearrange("(a p) d -> p a d", p=128))

                    for b in range(b0, b1):
                        tl = [(i, h, posh) for i, (b_, h, posh)
                              in enumerate(pr.tiles2[p]) if b_ == b]
                        agg = psA.tile([128, 320], F32, tag="agg")
                        for j, (gti, h, posh) in enumerate(tl):
                            gdsub, dt0 = gd[h]
                            adz = gdsub(posh - dt0)
                            oh = wpool.tile([128, 128], BF, tag="oh")
                            nc.vector.tensor_tensor(
                                out=oh[:],
                                in0=cs[("l", p)][:, gti:gti + 1]
                                    .to_broadcast([128, 128]),
                                in1=cw["iota"][:], op=OP.is_equal)
                            ohtp = psT.tile([128, 128], BF, tag="tp")
                            nc.tensor.transpose(out=ohtp[:], in_=oh[:],
                                                identity=cw["idbf"][:])
                            oht = wpool.tile([128, 128], BF, tag="oht")
                            nc.vector.tensor_copy(out=oht[:], in_=ohtp[:])
                            axe = psS.tile([128, 320], F32, tag="sel")
                            nc.tensor.matmul(out=axe[:], lhsT=oht[:],
                                             rhs=axx_g[:, b - b0, :320],
                                             start=True, stop=True)
                            hs = wpool.tile([128, 320], BF, tag="hs")
                            nc.vector.tensor_tensor(
                                out=hs[:], in0=axe[:],
                                in1=adz[:, :320], op=OP.add)
                            ht = wpool.tile([128, 320], BF, tag="ht")
                            nc.scalar.activation(out=ht[:], in_=hs[:],
                                                 func=AF.Tanh)
                            lm = wpool.tile([128, 320], BF, tag="lm")
                            nc.vector.tensor_tensor(
                                out=lm[:], in0=ht[:],
                                in1=cw[f"w2rep_{p}"][:], op=OP.mult)
                            lg = wpool.tile([128, c.C], F32, tag="lg")
                            nc.vector.tensor_reduce(
                                out=lg[:],
                                in_=lm[:].rearrange("q (c f) -> q c f", f=64),
                                axis=mybir.AxisListType.X, op=OP.add)
                            lgb = wpool.tile([128, c.C], F32, tag="lgb")
                            nc.vector.tensor_tensor(
                                out=lgb[:], in0=lg[:],
                                in1=cw[f"b2rep_{p}"][:], op=OP.add)
                            ex = wpool.tile([128, c.C], F32, tag="ex")
                            den = wpool.tile([128, 1], F32, tag="den")
                            nc.scalar.activation(out=ex[:], in_=lgb[:],
                                                 func=AF.Exp,
                                                 accum_out=den[:])
                            rec = wpool.tile([128, 1], F32, tag="rec")
                            nc.vector.reciprocal(out=rec[:], in_=den[:])
                            ws = wpool.tile([128, c.C], F32, tag="ws")
                            nc.scalar.activation(out=ws[:], in_=ex[:],
                                                 func=AF.Copy,
                                                 scale=rec[:, :1])
                            msg = wpool.tile([128, 320], BF, tag="msg")
                            nc.vector.tensor_tensor(
                                out=msg[:].rearrange("q (c f) -> q c f", f=64),
                                in0=adz[:, 320:640]
                                    .rearrange("q (c f) -> q c f", f=64),
                                in1=ws[:].to_broadcast([128, c.C, 64]),
                                op=OP.mult)
                            nc.tensor.matmul(out=agg[:], lhsT=oh[:],
                                             rhs=msg[:],
                                             start=(j == 0),
                                             stop=(j == len(tl) - 1))
                        u1p = wpool.tile([128, 320], F32, tag="u1p")
                        if tl:
                            nc.vector.tensor_scalar_mul(
                                out=u1p[:], in0=agg[:],
                                scalar1=cs[("i", p)][:, b:b + 1])
                        else:
                            nc.vector.memset(u1p[:], 0.0)
                        u1s = wpool.tile([128, 320], F32, tag="u1s")
                        nc.vector.tensor_tensor(
                            out=u1s[:], in0=u1p[:],
                            in1=axx_g[:, b - b0, 320:640], op=OP.add)
                        u1 = wpool.tile([128, 320], BF, tag="u1")
                        nc.scalar.activation(out=u1[:], in_=u1s[:],
                                             func=AF.Tanh)
                        u1t = wpool.tile([128, 3, 128], BF, tag="u1t")
                        for j in range(3):
                            colsj = 320 - 128 * j if j == 2 else 128
                            tp = psT.tile([128, 128], BF, tag="tp")
                            nc.tensor.transpose(
                                out=tp[:colsj, :],
                                in_=u1[:, 128 * j:128 * j + colsj],
                                identity=cw["idbf"][:])
                            nc.vector.tensor_copy(out=u1t[:colsj, j, :],
                                                  in_=tp[:colsj, :])
                        orow = opool.tile([128, 320], F32, tag="orow")
                        for gi, gcl in enumerate(G):
                            gp = 64 * len(gcl)
                            mm = psM.tile([128, 512], F32, tag="mlp")
                            nc.tensor.matmul(out=mm[:gp, :128],
                                             lhsT=cw[f"u2W_{p}_{gi}"][:],
                                             rhs=u1t[:gp, gi, :],
                                             start=True, stop=True)
                            u2 = wpool.tile([128, 128], BF, tag="u2")
                            nc.scalar.activation(
                                out=u2[:gp, :], in_=mm[:gp, :128],
                                func=AF.Tanh,
                                bias=cw[f"u2B_{p}_{gi}"][:gp, :])
                            tp = psT.tile([128, 128], BF, tag="tp")
                            nc.tensor.transpose(out=tp[:, :gp],
                                                in_=u2[:gp, :],
                                                identity=cw["idbf"][:gp, :gp])
                            nc.vector.tensor_copy(
                                out=orow[:, 128 * gi:128 * gi + gp],
                                in_=tp[:, :gp])
                        nc.sync.dma_start(
                            out=out.ap()[p, b * 128:(b + 1) * 128, :],
                            in_=orow[:])
    nc.compile()
    innames = ([f"xbs{p}" for p in range(c.P)] + ["andz"]
               + [f"g2d{p}_{h}" for p in range(c.P) for h in range(c.NH)]
               + [f"sloc{p}" for p in range(c.P)]
               + [f"icnt{p}" for p in range(c.P)] + wnames)
    return nc, innames


# ----------------------------------------------------------------------------
# in_maps
# ----------------------------------------------------------------------------

def l1_inmaps(cfg, pr, names):
    c = cfg
    maps = []
    for k in range(c.NC):
        m = {}
        for p in range(c.P):
            m[f"xb{p}"] = pr.xb[p]
            for q in range(c.NQ):
                m[f"gidx1_{p}_{q}"] = pr.gidx1[k][p][q]
            m[f"dloc1_{p}"] = pr.dloc1[k][p]
        for n in pr.W:
            m[n] = pr.W[n]
        maps.append({n: m[n] for n in names})
    return maps


def l2_inmaps(cfg, pr, andz_full, names):
    c = cfg
    maps = []
    for k in range(c.NC):
        m = {"andz": andz_full}
        for p in range(c.P):
            m[f"xbs{p}"] = pr.xb_slice[k][p]
            m[f"g2s{p}"] = pr.gidx2s[k][p]
            for h in range(c.NH):
                m[f"g2d{p}_{h}"] = pr.gidx2d[k][p][h]
            m[f"sloc{p}"] = pr.sloc2[k][p]
            m[f"icnt{p}"] = pr.invcnt[k][p]
        for n in pr.W:
            m[n] = pr.W[n]
        maps.append({n: m[n] for n in names})
    return maps


# ----------------------------------------------------------------------------
# public kernel()
# ----------------------------------------------------------------------------

def run(cfg, inputs, runner=None):
    """runner(nc, maps) -> list of per-core output dicts; default = HW SPMD."""
    pr = host_prep(cfg, inputs)

    nc1, in1 = build_l1(cfg, pr)
    maps1 = l1_inmaps(cfg, pr, in1)
    if runner is None:
        res1 = run_bass_kernel_spmd(nc1, maps1,
                                    core_ids=list(range(cfg.NC))).results
    else:
        res1 = runner(nc1, maps1)
    andz_full = np.concatenate([res1[k]["andz"] for k in range(cfg.NC)],
                               axis=1)

    nc2, in2 = build_l2(cfg, pr)
    maps2 = l2_inmaps(cfg, pr, andz_full, in2)
    if runner is None:
        res2 = run_bass_kernel_spmd(nc2, maps2,
                                    core_ids=list(range(cfg.NC))).results
    else:
        res2 = runner(nc2, maps2)

    out = np.concatenate([res2[k]["out"][:, :cfg.NK, :]
                          for k in range(cfg.NC)], axis=1)
    return np.ascontiguousarray(
        out.reshape(cfg.P, cfg.N, cfg.C, cfg.F).astype(np.float32))


def kernel(**inputs):
    return run(Cfg(), inputs)
